# revision 11
# baseline (speedup 1.0000x reference)
"""MEGAT layer (GAT w/ edge features) Trainium2 Bass kernel.

Full-input contract: kernel(**inputs) takes the complete arrays and returns
(out, alpha) exactly like the reference. Internally shards the batch dim
across 8 NeuronCores (32 batches each), runs one SPMD Bass program, and
concatenates the per-core results.

Per-batch dataflow (one of 32 per core):
  h|el|er = x_ext @ [W | W.al | W.ar ; bias|0]   (PE, bias folded via
            softmax-rows-sum-to-1: sum_j alpha*(h+b) = sum_j alpha*h + b)
  ert[8,128] = PE-transpose of er columns
  per head:
    er_bc = SEL_h.T @ ert          (PE K=8; SEL is a host-built selector)
    t     = Lrelu(er_bc + el_col)  (ACT, el added via per-partition bias)
    s     = t + e_h + negbig       (DVE; negbig = (adj<=0.5)*-1e9)
    numer = Exp(s), denom=row-sum  (ACT accum_out; masked entries -> 0)
    alpha = numer * (1/denom)      (DVE)
    out_h = alpha_h @ h_h          (PE transpose + PE matmul)
  out = elu(out + x) via relu(v) + min(exp(v)-1, 0)
"""

import sys

for _p in ("/opt/trn_rl_repo",):
    if _p not in sys.path:
        sys.path.insert(0, _p)

import numpy as np

import concourse.bass as bass
import concourse.bacc as bacc
import concourse.mybir as mybir
import concourse.tile as tile
from concourse.masks import make_identity
from concourse.bass_utils import run_bass_kernel_spmd

B, N, H, F = 256, 128, 8, 256
FH = F // H  # 32
NCORES = 8
BL = B // NCORES  # 32 batches per core
NEG_SLOPE = 0.2
NEG_BIG = -1.0e9
FP = mybir.dt.float32

_CACHE = {}


def _build_program(bl=BL, finalize=True):
    """Build the per-core Bass program (inputs are the per-core shard)."""
    BLv = bl
    nc = bacc.Bacc("TRN2", target_bir_lowering=False, debug=False,
                   num_devices=NCORES)

    adj_d = nc.declare_dram_parameter("adj", [BLv, N, N], FP, isOutput=False)
    x_d = nc.declare_dram_parameter("x", [BLv, N, F], FP, isOutput=False)
    e_d = nc.declare_dram_parameter("e", [BLv, H, N, N], FP, isOutput=False)
    # wext = [W | W@AL | W@AR] as [2, 128, 272] (K split into two 128-chunks)
    wext_d = nc.declare_dram_parameter("wext", [2, 128, F + 2 * H], FP, isOutput=False)
    # biasext = [bias | zeros(16)] -> K=1 matmul row
    biasext_d = nc.declare_dram_parameter("biasext", [1, F + 2 * H], FP, isOutput=False)
    # sel[k, h*128 + i] = (k == h): selector for the er row-broadcast matmul
    sel_d = nc.declare_dram_parameter("sel", [H, H * N], FP, isOutput=False)

    out_d = nc.declare_dram_parameter("out", [BLv, N, F], FP, isOutput=True)
    alpha_d = nc.declare_dram_parameter("alpha", [BLv, H, N, N], FP, isOutput=True)

    FE = F + 2 * H  # 272

    with tile.TileContext(nc) as tc:
        with (
            tc.tile_pool(name="consts", bufs=1) as consts,
            tc.tile_pool(name="xin", bufs=3) as xin,
            tc.tile_pool(name="ein", bufs=3) as ein,
            tc.tile_pool(name="adjin", bufs=3) as adjin,
            tc.tile_pool(name="hp", bufs=3) as hp,
            tc.tile_pool(name="alph", bufs=3) as alph,
            tc.tile_pool(name="small", bufs=3) as small,
            tc.tile_pool(name="perhead", bufs=6) as perhead,
            tc.tile_pool(name="epil", bufs=3) as epil,
            tc.tile_pool(name="ps_mid", bufs=2, space="PSUM") as ps_mid,
            tc.tile_pool(name="ps_out", bufs=2, space="PSUM") as ps_out,
            tc.tile_pool(name="ps_erbc", bufs=2, space="PSUM") as ps_erbc,
            tc.tile_pool(name="ps_at", bufs=2, space="PSUM") as ps_at,
        ):
            identity = consts.tile([128, 128], FP)
            make_identity(nc, identity)
            wext_sb = consts.tile([128, 2, FE], FP)
            nc.sync.dma_start(out=wext_sb, in_=wext_d[:].rearrange("t p f -> p t f"))
            biasext_sb = consts.tile([1, FE], FP)
            nc.sync.dma_start(out=biasext_sb, in_=biasext_d[:])
            sel_sb = consts.tile([H, H * N], FP)
            nc.sync.dma_start(out=sel_sb, in_=sel_d[:])
            ones_row = consts.tile([1, 128], FP)
            nc.vector.memset(ones_row, 1.0)

            for b in range(BLv):
                x_t = xin.tile([128, F], FP)
                nc.sync.dma_start(out=x_t, in_=x_d[b])
                adj_t = adjin.tile([128, N], FP)
                nc.sync.dma_start(out=adj_t, in_=adj_d[b])
                e_t = ein.tile([128, H, N], FP)
                nc.sync.dma_start(out=e_t, in_=e_d[b].rearrange("h i j -> i h j"))

                # negbig = (adj <= 0.5) * -1e9  (additive mask)
                negbig = adjin.tile([128, N], FP, tag="negbig")
                nc.vector.tensor_scalar(
                    out=negbig, in0=adj_t, scalar1=0.5, scalar2=NEG_BIG,
                    op0=mybir.AluOpType.is_le, op1=mybir.AluOpType.mult,
                )

                # x^T (two 128x128 PE transposes) for the h matmul
                xT_ps = ps_mid.tile([128, 2, 128], FP, tag="scratch")
                for t in range(2):
                    nc.tensor.transpose(xT_ps[:, t, :], x_t[:, t * 128:(t + 1) * 128], identity)
                xT_sb = small.tile([128, 2, 128], FP, tag="xT")
                nc.vector.tensor_copy(out=xT_sb, in_=xT_ps)

                # h(+bias) | el | er = x_ext @ [W|WAL|WAR ; biasext]
                hel_ps = ps_out.tile([128, FE], FP, tag="helout")
                nc.tensor.matmul(hel_ps, lhsT=xT_sb[:, 0, :], rhs=wext_sb[:, 0, :],
                                 start=True, stop=False)
                nc.tensor.matmul(hel_ps, lhsT=xT_sb[:, 1, :], rhs=wext_sb[:, 1, :],
                                 start=False, stop=False)
                nc.tensor.matmul(hel_ps, lhsT=ones_row, rhs=biasext_sb,
                                 start=False, stop=True)
                h_sb = hp.tile([128, FE], FP)
                nc.vector.tensor_copy(out=h_sb, in_=hel_ps)

                # er^T [8, 128] via PE transpose of the er columns
                ert_ps = ps_mid.tile([H, 128], FP, tag="scratch")
                nc.tensor.transpose(ert_ps, h_sb[:, F + H:F + 2 * H], identity)
                ert_sb = small.tile([H, 128], FP, tag="ert")
                nc.vector.tensor_copy(out=ert_sb, in_=ert_ps)

                # e_m = e + negbig (broadcast over heads) on idle gpsimd
                e_m = ein.tile([128, H, N], FP, tag="em")
                negbig_b = bass.AP(
                    tensor=negbig.tensor, offset=negbig.offset,
                    ap=[negbig.ap[0], [0, H], negbig.ap[1]],
                )
                nc.gpsimd.tensor_add(e_m, e_t, negbig_b)

                alpha_t = alph.tile([128, H, N], FP)
                denom = small.tile([128, H], FP, tag="denom")
                rd = small.tile([128, H], FP, tag="rd")
                out_ps = ps_out.tile([128, F], FP, tag="helout")

                for h in range(H):
                    # er_bc[i,j] = er_j[h] (selector matmul broadcast)
                    erbc_ps = ps_erbc.tile([128, N], FP)
                    nc.tensor.matmul(erbc_ps, lhsT=sel_sb[:, h * N:(h + 1) * N],
                                     rhs=ert_sb, start=True, stop=True)
                    # t = lrelu(er_bc + el_i)  (el via per-partition bias)
                    t_sb = perhead.tile([128, N], FP, tag="t")
                    nc.scalar.activation(out=t_sb, in_=erbc_ps,
                                         func=mybir.ActivationFunctionType.Prelu,
                                         bias=h_sb[:, F + h:F + h + 1],
                                         alpha=NEG_SLOPE)
                    # s = t + e + negbig
                    s_sb = perhead.tile([128, N], FP, tag="s")
                    nc.gpsimd.tensor_add(s_sb, t_sb, e_m[:, h, :])
                    # numer = exp(s), denom = row-sum (masked entries exp -> 0)
                    nc.scalar.activation(out=alpha_t[:, h, :], in_=s_sb,
                                         func=mybir.ActivationFunctionType.Exp,
                                         accum_out=denom[:, h:h + 1])
                    nc.vector.reciprocal(out=rd[:, h:h + 1], in_=denom[:, h:h + 1])
                    nc.vector.tensor_scalar_mul(out=alpha_t[:, h, :],
                                                in0=alpha_t[:, h, :],
                                                scalar1=rd[:, h:h + 1])
                    # alpha^T for the aggregation matmul
                    at_ps = ps_at.tile([128, N], FP)
                    nc.tensor.transpose(at_ps, alpha_t[:, h, :], identity)
                    at_sb = perhead.tile([128, N], FP, tag="at")
                    nc.vector.tensor_copy(out=at_sb, in_=at_ps)
                    # out[:, h*32:(h+1)*32] = alpha_h @ h_h
                    nc.tensor.matmul(out_ps[:, h * FH:(h + 1) * FH], lhsT=at_sb,
                                     rhs=h_sb[:, h * FH:(h + 1) * FH],
                                     start=True, stop=True)

                nc.sync.dma_start(out=alpha_d[b].rearrange("h i j -> i h j"),
                                  in_=alpha_t)

                # epilogue: v = out + x ; out = relu(v) + min(exp(v)-1, 0)
                v_sb = epil.tile([128, F], FP, tag="v")
                nc.vector.tensor_add(v_sb, out_ps, x_t)
                ev_sb = epil.tile([128, F], FP, tag="ev")
                nc.scalar.activation(out=ev_sb, in_=v_sb,
                                     func=mybir.ActivationFunctionType.Exp)
                nc.vector.tensor_scalar(out=ev_sb, in0=ev_sb, scalar1=-1.0,
                                        scalar2=0.0, op0=mybir.AluOpType.add,
                                        op1=mybir.AluOpType.min)
                f_sb = epil.tile([128, F], FP, tag="f")
                nc.vector.scalar_tensor_tensor(out=f_sb, in0=v_sb, scalar=0.0,
                                               in1=ev_sb,
                                               op0=mybir.AluOpType.max,
                                               op1=mybir.AluOpType.add)
                nc.sync.dma_start(out=out_d[b], in_=f_sb)

    if finalize:
        nc.finalize()
    else:
        nc.compile()
    return nc


def _get_program():
    if "nc" not in _CACHE:
        _CACHE["nc"] = _build_program()
    return _CACHE["nc"]


def kernel(adj, x, e, W, attn_l, attn_r, bias, _trace=False, **_trace_kwargs):
    adj = np.ascontiguousarray(adj, dtype=np.float32)
    x = np.ascontiguousarray(x, dtype=np.float32)
    e = np.ascontiguousarray(e, dtype=np.float32)
    W = np.asarray(W, dtype=np.float32)
    attn_l = np.asarray(attn_l, dtype=np.float32)
    attn_r = np.asarray(attn_r, dtype=np.float32)
    bias = np.asarray(bias, dtype=np.float32)

    # Host-side prep: WAL[c,h] = sum_f W[c, h*FH+f] * attn_l[h,f]; same for WAR
    W_r = W.reshape(F, H, FH)
    WAL = np.einsum("chf,hf->ch", W_r, attn_l).astype(np.float32)
    WAR = np.einsum("chf,hf->ch", W_r, attn_r).astype(np.float32)
    wext = np.concatenate([W, WAL, WAR], axis=1).reshape(2, 128, F + 2 * H)
    wext = np.ascontiguousarray(wext)
    biasext = np.zeros((1, F + 2 * H), dtype=np.float32)
    biasext[0, :F] = bias
    sel = np.zeros((H, H, N), dtype=np.float32)
    for h in range(H):
        sel[h, h, :] = 1.0
    sel = sel.reshape(H, H * N)

    nc = _get_program()
    in_maps = []
    for c in range(NCORES):
        sl = slice(c * BL, (c + 1) * BL)
        in_maps.append({
            "adj": adj[sl], "x": x[sl], "e": e[sl],
            "wext": wext, "biasext": biasext, "sel": sel,
        })
    res = run_bass_kernel_spmd(nc, in_maps, list(range(NCORES)),
                               trace=_trace, **_trace_kwargs)
    outs = np.concatenate([res.results[c]["out"] for c in range(NCORES)], axis=0)
    alphas = np.concatenate([res.results[c]["alpha"] for c in range(NCORES)], axis=0)
    if _trace:
        return (outs, alphas), res
    return outs, alphas
